# revision 2
# baseline (speedup 1.0000x reference)
"""Depthwise 3x3 conv on 8 trn2 NeuronCores — v5 (host-padded, multi-engine).

Host transposes x to [b, c, 114*114] fp16 with SAME-padding zeros baked in,
so the device loads each channel tile with ONE contiguous DMA (no DVE
spread, no guard memsets, no wraparound fixups).

The 9 taps are split across engines:
  - PE (7 taps): diag-matmul accumulation in PSUM. Tap-outer loop per
    super-chunk (4 PSUM banks x 448 cols = 16 rows) amortizes LDWEIGHTS.
  - ScalarE: evacuates 4 PSUM banks per activation (Identity + per-channel
    bias AP) into the fp16 out tile.
  - DVE (2 taps): fused scalar_tensor_tensor MACs (out += w_c * x_shifted)
    over the full image tile, chained after the evacuated partial.

c=192 = 128 + 64: per image pair (A, B), three 128-partition tiles:
  g=0: A channels 0..127, g=1: B channels 0..127,
  g=2: A channels 128..191 | B channels 128..191 (packed 64+64).
"""
import dataclasses

import numpy as np

import concourse.bacc as bacc
import concourse.mybir as mybir
from concourse.bass_utils import run_bass_kernel_spmd
from concourse.tile import TileContext

F32 = mybir.dt.float32
F16 = mybir.dt.float16
NPF16 = np.float16

B, H, W, C = 32, 112, 112, 192
N_CORES = 8
B_SH = B // N_CORES
NPIX = H * W                  # 12544
PW2 = W + 2                   # padded pitch 114
NPAD2 = (H + 2) * PW2         # 12996
CH_ROWS = 4                   # output rows per PSUM bank
CHW = CH_ROWS * W             # 448 fp32 per bank
SC_CH = 4                     # banks per super-chunk
SC_ROWS = SC_CH * CH_ROWS     # 16
SCW = SC_ROWS * W             # 1792
NSC = H // SC_ROWS            # 7

# tap split: (dh, dw) with input row = out_row + dh, col = out_col + dw
TAPS_PE = [(0, 0), (0, 1), (0, 2), (1, 1), (2, 0), (2, 1), (2, 2)]
TAPS_DVE = [(1, 0), (1, 2)]


def _ap3(t, offset, s0, n0, s1, n1):
    """3D free AP [[s0,n0],[s1,n1]] at free offset within tile t."""
    sl = t[:, offset:offset + 1]
    return dataclasses.replace(sl, ap=[sl.ap[0], [s0, n0], [s1, n1]])


def _emit_g(nc, wd, wc, xp, opool, pspool, g, y_a, y_b):
    """Conv + evac + DVE taps + store for one 128-partition tile g."""
    gsel = 1 if g == 2 else 0
    ot = opool.tile([128, NPIX], F16, tag="ot", name="ot")
    for sc in range(NSC):
        r0 = sc * SC_ROWS
        ps = pspool.tile([128, 2048], F32, tag="ps", name="ps")
        for ti, (dh, dw) in enumerate(TAPS_PE):
            for ck in range(SC_CH):
                off = (r0 + ck * CH_ROWS + dh) * PW2 + dw
                nc.tensor.matmul(
                    ps[:, ck * 512:ck * 512 + CHW],
                    wd[:, (ti * 2 + gsel) * 128:(ti * 2 + gsel + 1) * 128],
                    _ap3(xp, off, PW2, CH_ROWS, 1, W),
                    start=(ti == 0), stop=(ti == len(TAPS_PE) - 1))
        # evac 4 banks in one activation, bias folded in
        nc.scalar.activation(
            _ap3(ot, sc * SCW, CHW, SC_CH, 1, CHW),
            _ap3(ps, 0, 512, SC_CH, 1, CHW),
            mybir.ActivationFunctionType.Identity,
            bias=wc[:, 4 + gsel:5 + gsel])
    for ti, (dh, dw) in enumerate(TAPS_DVE):
        nc.vector.scalar_tensor_tensor(
            _ap3(ot, 0, W, H, 1, W),
            _ap3(xp, dh * PW2 + dw, PW2, H, 1, W),
            wc[:, ti * 2 + gsel:ti * 2 + gsel + 1],
            _ap3(ot, 0, W, H, 1, W),
            mybir.AluOpType.mult, mybir.AluOpType.add)
    if g < 2:
        y = y_a if g == 0 else y_b
        nc.sync.dma_start(out=y[0:128, :], in_=ot[:, :])
    else:
        nc.sync.dma_start(out=y_a[128:192, :], in_=ot[0:64, :])
        nc.sync.dma_start(out=y_b[128:192, :], in_=ot[64:128, :])


def _load_g(nc, g, x_a, x_b, xps):
    if g == 0:
        nc.sync.dma_start(out=xps[0][:, :], in_=x_a[0:128, :])
    elif g == 1:
        nc.sync.dma_start(out=xps[1][:, :], in_=x_b[0:128, :])
    else:
        nc.sync.dma_start(out=xps[2][0:64, :], in_=x_a[128:192, :])
        nc.sync.dma_start(out=xps[2][64:128, :], in_=x_b[128:192, :])


def _build_module(b_sh=B_SH):
    nc = bacc.Bacc("TRN2")
    x = nc.dram_tensor("x", [b_sh, C, NPAD2], F16, kind="ExternalInput")
    wdiag = nc.dram_tensor("wdiag", [128, len(TAPS_PE) * 2 * 128], F16,
                           kind="ExternalInput")
    wcolb = nc.dram_tensor("wcolb", [128, 6], F32, kind="ExternalInput")
    y = nc.dram_tensor("y", [b_sh, C, NPIX], F16, kind="ExternalOutput")

    with TileContext(nc) as tc:
        with (
            tc.tile_pool(name="const", bufs=1) as cpool,
            tc.tile_pool(name="xp", bufs=1) as xpool,
            tc.tile_pool(name="outp", bufs=3) as opool,
            tc.tile_pool(name="psum", bufs=2, space="PSUM") as pspool,
        ):
            wd = cpool.tile([128, len(TAPS_PE) * 2 * 128], F16,
                            tag="wd", name="wd")
            nc.sync.dma_start(out=wd[:, :], in_=wdiag[:, :])
            wc = cpool.tile([128, 6], F32, tag="wc", name="wc")
            nc.sync.dma_start(out=wc[:, :], in_=wcolb[:, :])
            xps = [xpool.tile([128, NPAD2], F16, tag=f"xp_{g}",
                              name=f"xp_{g}") for g in range(3)]
            npair = b_sh // 2
            for g in range(3):
                _load_g(nc, g, x[0], x[1], xps)
            for pair in range(npair):
                a, b = 2 * pair, 2 * pair + 1
                for g in range(3):
                    _emit_g(nc, wd, wc, xps[g], opool, pspool, g, y[a], y[b])
                    if pair + 1 < npair:
                        _load_g(nc, g, x[a + 2], x[b + 2], xps)
    nc.compile()
    return nc


def _build_timing_module(iters=8):
    nc = bacc.Bacc("TRN2")
    x = nc.dram_tensor("xg", [2, C, NPAD2], F16)
    y = nc.dram_tensor("yg", [2, C, NPIX], F16)
    yo = nc.dram_tensor("yo", [1, 8], F32, kind="ExternalOutput")

    with TileContext(nc) as tc:
        with (
            tc.tile_pool(name="const", bufs=1) as cpool,
            tc.tile_pool(name="xp", bufs=1) as xpool,
            tc.tile_pool(name="outp", bufs=3) as opool,
            tc.tile_pool(name="psum", bufs=2, space="PSUM") as pspool,
        ):
            wd = cpool.tile([128, len(TAPS_PE) * 2 * 128], F16,
                            tag="wd", name="wd")
            nc.vector.memset(wd[:, :], 0.01)
            wc = cpool.tile([128, 6], F32, tag="wc", name="wc")
            nc.vector.memset(wc[:, :], 0.01)
            xps = [xpool.tile([128, NPAD2], F16, tag=f"xp_{g}",
                              name=f"xp_{g}") for g in range(3)]
            zt = opool.tile([128, NPIX], F16, tag="ot", name="zt")
            nc.vector.memset(zt[:, :], 0.5)
            for img in range(2):
                nc.sync.dma_start(out=x[img, 0:128, 0:NPIX], in_=zt[:, :])
                nc.sync.dma_start(out=x[img, 64:192, 0:NPIX], in_=zt[:, :])
            for g in range(3):
                _load_g(nc, g, x[0], x[1], xps)
            with tc.For_i(0, iters) as _:
                # one iter = one image pair, steady-state JIT refill
                for g in range(3):
                    _emit_g(nc, wd, wc, xps[g], opool, pspool, g, y[0], y[1])
                    _load_g(nc, g, x[0], x[1], xps)
            of = opool.tile([1, 8], F32, tag="of", name="of")
            nc.vector.memset(of[:, :], 0.0)
            nc.sync.dma_start(out=yo[:, :], in_=of[:1, :8])
    nc.compile()
    return nc


def _host_consts(wk, bk):
    """wk [3,3,1,192], bk [192] -> (wdiag f16, wcolb f32)."""
    npe = len(TAPS_PE)
    wd = np.zeros((128, npe * 2 * 128), np.float32)
    for i, (dh, dw) in enumerate(TAPS_PE):
        w_t = wk[dh, dw, 0]  # [192]
        wd[:, (i * 2) * 128:(i * 2 + 1) * 128] = np.diag(w_t[0:128])
        wd[:, (i * 2 + 1) * 128:(i * 2 + 2) * 128] = np.diag(
            np.concatenate([w_t[128:192], w_t[128:192]]))
    wc = np.zeros((128, 6), np.float32)
    for i, (dh, dw) in enumerate(TAPS_DVE):
        w_t = wk[dh, dw, 0]
        wc[:, i * 2] = w_t[0:128]
        wc[:, i * 2 + 1] = np.concatenate([w_t[128:192], w_t[128:192]])
    wc[:, 4] = bk[0:128]
    wc[:, 5] = np.concatenate([bk[128:192], bk[128:192]])
    return wd.astype(NPF16), wc.astype(np.float32)


_NC_CACHE = {}


def kernel(x, w, b):
    x = np.asarray(x, dtype=np.float32)
    wk = np.asarray(w, dtype=np.float32)
    bk = np.asarray(b, dtype=np.float32)
    assert x.shape == (B, H, W, C), x.shape

    if "nc" not in _NC_CACHE:
        _NC_CACHE["nc"] = _build_module()
    nc = _NC_CACHE["nc"]

    xt = np.zeros((B, C, H + 2, PW2), NPF16)
    xt[:, :, 1:H + 1, 1:W + 1] = x.astype(NPF16).transpose(0, 3, 1, 2)
    xt = xt.reshape(B, C, NPAD2)
    wdiag, wcolb = _host_consts(wk, bk)
    in_maps = []
    for core in range(N_CORES):
        sh = xt[core * B_SH:(core + 1) * B_SH]
        in_maps.append({"x": np.ascontiguousarray(sh), "wdiag": wdiag,
                        "wcolb": wcolb})
    res = run_bass_kernel_spmd(nc, in_maps, core_ids=list(range(N_CORES)))
    out = np.empty((B, C, NPIX), np.float32)
    for core in range(N_CORES):
        out[core * B_SH:(core + 1) * B_SH] = res.results[core]["y"]
    return np.ascontiguousarray(
        out.reshape(B, C, H, W).transpose(0, 2, 3, 1))


# revision 4
# speedup vs baseline: 1.0666x; 1.0666x over previous
"""Depthwise 3x3 conv on 8 trn2 NeuronCores — v6 (host-padded baseline).

Host transposes x to [b, c, 114*114] fp16 with SAME-padding zeros baked in,
so each channel tile loads with ONE contiguous DMA: the baseline's staging
tile, DVE spread pass, and guard memsets all disappear, and the per-channel
bias is folded into the PSUM-evacuation activation (Identity + bias AP).

Compute structure is the proven baseline pattern: per 448-col PSUM chunk,
tap-inner diag-matmul accumulation on the PE; ScalarE evacuates each chunk
into a 7-chunk fp16 store tile. Optional TAPS_DVE moves trailing taps to
DVE as fused scalar_tensor_tensor MACs per store tile.

c=192 = 128 + 64: per image pair (A, B), three 128-partition tiles:
  g=0: A channels 0..127, g=1: B channels 0..127,
  g=2: A channels 128..191 | B channels 128..191 (packed 64+64).
"""
import dataclasses

import numpy as np

import concourse.bacc as bacc
import concourse.mybir as mybir
from concourse.bass_utils import run_bass_kernel_spmd
from concourse.tile import TileContext

F32 = mybir.dt.float32
F16 = mybir.dt.float16
NPF16 = np.float16

B, H, W, C = 32, 112, 112, 192
N_CORES = 8
B_SH = B // N_CORES
NPIX = H * W                  # 12544
PW2 = W + 2                   # padded pitch 114
NPAD2 = (H + 2) * PW2         # 12996
CHW = 448                     # 4 rows per PSUM chunk
QCH = 7                       # chunks per store tile
QFREE = QCH * CHW             # 3136

TAPS_PE = [(0, 0), (0, 1), (0, 2), (1, 0), (1, 1), (1, 2), (2, 0)]
TAPS_DVE = [(2, 1), (2, 2)]


def _ap3(t, offset, s0, n0, s1, n1):
    sl = t[:, offset:offset + 1]
    return dataclasses.replace(sl, ap=[sl.ap[0], [s0, n0], [s1, n1]])


def _emit_g(nc, wd, wc, xp, opool, pspool, g, y_a, y_b):
    gsel = 1 if g == 2 else 0
    for q in range(4):
        outq = opool.tile([128, QFREE], F16, tag="outq", name="outq")
        for cc in range(QCH):
            r0 = (q * QCH + cc) * 4
            ps = pspool.tile([128, CHW], F32, tag="ps", name="ps")
            for ti, (dh, dw) in enumerate(TAPS_PE):
                nc.tensor.matmul(
                    ps[:, :],
                    wd[:, (ti * 2 + gsel) * 128:(ti * 2 + gsel + 1) * 128],
                    _ap3(xp, (r0 + dh) * PW2 + dw, PW2, 4, 1, W),
                    start=(ti == 0), stop=(ti == len(TAPS_PE) - 1))
            nc.scalar.activation(
                outq[:, cc * CHW:(cc + 1) * CHW], ps[:, :],
                mybir.ActivationFunctionType.Identity,
                bias=wc[:, 4 + gsel:5 + gsel])
        for ti, (dh, dw) in enumerate(TAPS_DVE):
            nc.vector.scalar_tensor_tensor(
                _ap3(outq, 0, W, 28, 1, W),
                _ap3(xp, (q * 28 + dh) * PW2 + dw, PW2, 28, 1, W),
                wc[:, ti * 2 + gsel:ti * 2 + gsel + 1],
                _ap3(outq, 0, W, 28, 1, W),
                mybir.AluOpType.mult, mybir.AluOpType.add)
        q0 = q * QFREE
        if g < 2:
            y = y_a if g == 0 else y_b
            nc.sync.dma_start(out=y[0:128, q0:q0 + QFREE], in_=outq[:, :])
        else:
            nc.sync.dma_start(out=y_a[128:192, q0:q0 + QFREE],
                              in_=outq[0:64, :])
            nc.sync.dma_start(out=y_b[128:192, q0:q0 + QFREE],
                              in_=outq[64:128, :])


def _load_g(nc, g, x_a, x_b, xps):
    if g == 0:
        nc.sync.dma_start(out=xps[0][:, :], in_=x_a[0:128, :])
    elif g == 1:
        nc.sync.dma_start(out=xps[1][:, :], in_=x_b[0:128, :])
    else:
        nc.sync.dma_start(out=xps[2][0:64, :], in_=x_a[128:192, :])
        nc.sync.dma_start(out=xps[2][64:128, :], in_=x_b[128:192, :])


def _make_pools(tc):
    return (
        tc.tile_pool(name="const", bufs=1),
        tc.tile_pool(name="xp", bufs=1),
        tc.tile_pool(name="outp", bufs=3),
        tc.tile_pool(name="psum", bufs=4, space="PSUM"),
    )


def _build_module(b_sh=B_SH):
    nc = bacc.Bacc("TRN2")
    x = nc.dram_tensor("x", [b_sh, C, NPAD2], F16, kind="ExternalInput")
    wdiag = nc.dram_tensor("wdiag", [128, 9 * 2 * 128], F16,
                           kind="ExternalInput")
    wcolb = nc.dram_tensor("wcolb", [128, 6], F32, kind="ExternalInput")
    y = nc.dram_tensor("y", [b_sh, C, NPIX], F16, kind="ExternalOutput")

    with TileContext(nc) as tc:
        cp, xpp, opp, psp = _make_pools(tc)
        with cp as cpool, xpp as xpool, opp as opool, psp as pspool:
            wd = cpool.tile([128, 9 * 2 * 128], F16, tag="wd", name="wd")
            nc.sync.dma_start(out=wd[:, :], in_=wdiag[:, :])
            wc = cpool.tile([128, 6], F32, tag="wc", name="wc")
            nc.sync.dma_start(out=wc[:, :], in_=wcolb[:, :])
            xps = [xpool.tile([128, NPAD2], F16, tag=f"xp_{g}",
                              name=f"xp_{g}") for g in range(3)]
            npair = b_sh // 2
            for g in range(3):
                _load_g(nc, g, x[0], x[1], xps)
            for pair in range(npair):
                a, b = 2 * pair, 2 * pair + 1
                for g in range(3):
                    _emit_g(nc, wd, wc, xps[g], opool, pspool, g, y[a], y[b])
                    if pair + 1 < npair:
                        _load_g(nc, g, x[a + 2], x[b + 2], xps)
    nc.compile()
    return nc


def _build_timing_module(iters=8):
    nc = bacc.Bacc("TRN2")
    x = nc.dram_tensor("xg", [2, C, NPAD2], F16)
    y = nc.dram_tensor("yg", [2, C, NPIX], F16)
    yo = nc.dram_tensor("yo", [1, 8], F32, kind="ExternalOutput")

    with TileContext(nc) as tc:
        cp, xpp, opp, psp = _make_pools(tc)
        with cp as cpool, xpp as xpool, opp as opool, psp as pspool:
            wd = cpool.tile([128, 9 * 2 * 128], F16, tag="wd", name="wd")
            nc.vector.memset(wd[:, :], 0.01)
            wc = cpool.tile([128, 6], F32, tag="wc", name="wc")
            nc.vector.memset(wc[:, :], 0.01)
            xps = [xpool.tile([128, NPAD2], F16, tag=f"xp_{g}",
                              name=f"xp_{g}") for g in range(3)]
            zt = opool.tile([128, QFREE], F16, tag="outq", name="zt")
            nc.vector.memset(zt[:, :], 0.5)
            for img in range(2):
                for q in range(4):
                    nc.sync.dma_start(
                        out=x[img, 0:128, q * QFREE:(q + 1) * QFREE],
                        in_=zt[:, :])
                    nc.sync.dma_start(
                        out=x[img, 64:192, q * QFREE:(q + 1) * QFREE],
                        in_=zt[:, :])
            for g in range(3):
                _load_g(nc, g, x[0], x[1], xps)
            with tc.For_i(0, iters) as _:
                for g in range(3):
                    _emit_g(nc, wd, wc, xps[g], opool, pspool, g, y[0], y[1])
                    _load_g(nc, g, x[0], x[1], xps)
            of = opool.tile([1, 8], F32, tag="of", name="of")
            nc.vector.memset(of[:, :], 0.0)
            nc.sync.dma_start(out=yo[:, :], in_=of[:1, :8])
    nc.compile()
    return nc


def _host_consts(wk, bk):
    """wk [3,3,1,192], bk [192] -> (wdiag f16, wcolb f32)."""
    wd = np.zeros((128, 9 * 2 * 128), np.float32)  # sized for up to 9 taps
    for i, (dh, dw) in enumerate(TAPS_PE):
        w_t = wk[dh, dw, 0]
        wd[:, (i * 2) * 128:(i * 2 + 1) * 128] = np.diag(w_t[0:128])
        wd[:, (i * 2 + 1) * 128:(i * 2 + 2) * 128] = np.diag(
            np.concatenate([w_t[128:192], w_t[128:192]]))
    wc = np.zeros((128, 6), np.float32)
    for i, (dh, dw) in enumerate(TAPS_DVE):
        w_t = wk[dh, dw, 0]
        wc[:, i * 2] = w_t[0:128]
        wc[:, i * 2 + 1] = np.concatenate([w_t[128:192], w_t[128:192]])
    wc[:, 4] = bk[0:128]
    wc[:, 5] = np.concatenate([bk[128:192], bk[128:192]])
    return wd.astype(NPF16), wc.astype(np.float32)


_NC_CACHE = {}


def kernel(x, w, b):
    x = np.asarray(x, dtype=np.float32)
    wk = np.asarray(w, dtype=np.float32)
    bk = np.asarray(b, dtype=np.float32)
    assert x.shape == (B, H, W, C), x.shape

    if "nc" not in _NC_CACHE:
        _NC_CACHE["nc"] = _build_module()
    nc = _NC_CACHE["nc"]

    xt = np.zeros((B, C, H + 2, PW2), NPF16)
    xt[:, :, 1:H + 1, 1:W + 1] = x.astype(NPF16).transpose(0, 3, 1, 2)
    xt = xt.reshape(B, C, NPAD2)
    wdiag, wcolb = _host_consts(wk, bk)
    in_maps = []
    for core in range(N_CORES):
        sh = xt[core * B_SH:(core + 1) * B_SH]
        in_maps.append({"x": np.ascontiguousarray(sh), "wdiag": wdiag,
                        "wcolb": wcolb})
    res = run_bass_kernel_spmd(nc, in_maps, core_ids=list(range(N_CORES)))
    out = np.empty((B, C, NPIX), np.float32)
    for core in range(N_CORES):
        out[core * B_SH:(core + 1) * B_SH] = res.results[core]["y"]
    return np.ascontiguousarray(
        out.reshape(B, C, H, W).transpose(0, 2, 3, 1))
